# revision 1
# baseline (speedup 1.0000x reference)
"""Multi-Head Latent Attention on 8 Trainium2 NeuronCores.

Sharding: core c = (batch b = c//4) x (head-group g = c%4, 4 heads each).
Each core computes the down-projections for its batch (replicated within
the 4-core batch group), up-projections/rope/attention for its 4 heads,
and a partial output projection. Host sums the 4 partials per batch and
adds the output bias (plus the value-up bias folded through out_w, which
is exact because softmax rows sum to 1).

All on-device layouts are feature-major ("transposed"): x^T, kvq_c^T,
K^T, Q^T, ctx^T, out^T. This makes every matmul contraction land on the
partition axis with zero transposes. Scores are computed as
scores^T[k, q] so that probs^T feeds the context matmul directly; the
softmax denominator comes from a ones-vector matmul (partition-axis sum
on the PE), and exp is applied without max-subtraction (scores for this
problem are in [-1, 1], verified offline).

Rope is applied via the "swapped-weight" identity:
  rot(Wx + b) = cos .* (Wx + b) + sin .* (W_swap x + b_swap)
with W_swap column pairs (w_{2i}, w_{2i+1}) -> (-w_{2i+1}, w_{2i}), which
keeps everything partition-aligned (no cross-partition reads).
"""

import numpy as np
import ml_dtypes

import concourse.bass as bass
import concourse.mybir as mybir
from concourse.tile import TileContext
from concourse.bass_utils import run_bass_kernel_spmd

F32 = mybir.dt.float32
BF16 = mybir.dt.bfloat16
AF = mybir.ActivationFunctionType
BF = ml_dtypes.bfloat16

HIDDEN = 2048
NUM_HEADS = 16
HEAD_DIM = 128
KV_C = 512
Q_C = 1536
ROPE_DIM = 64
B, S = 2, 2048

P = 128
NH = 4          # heads per core
SC = 512        # free-dim chunk for projections / q-chunks
NKT = HIDDEN // P       # 16 k-tiles of the down projection
NMD = HIDDEN // P       # 16 output chunks of the down projection (kv+q)
SCALE = float(1.0 / np.sqrt(HEAD_DIM + ROPE_DIM))
NEG = -1.0e5


def _split_waits(nc, maxw=1):
    """This container's walrus accepts at most one sem-wait per instruction;
    move excess waits onto same-engine NOPs inserted immediately before."""
    for fn in nc.m.functions:
        for bb in fn.blocks:
            newlist = []
            for ins in bb.instructions:
                si = ins.sync_info
                if si is not None and si.on_wait is not None and len(si.on_wait) > maxw:
                    waits = list(si.on_wait)
                    extra, keep = waits[:-maxw], waits[-maxw:]
                    for k, i in enumerate(range(0, len(extra), maxw)):
                        nop = mybir.InstNoOp(
                            name=f"{ins.name}-waitsplit-{k}", ins=[], outs=[]
                        )
                        nop.engine = ins.engine
                        nop.sync_info = mybir.SyncInfo(
                            on_wait=extra[i : i + maxw], on_update=[]
                        )
                        newlist.append(nop)
                    ins.sync_info = mybir.SyncInfo(
                        on_wait=keep, on_update=list(si.on_update or [])
                    )
                newlist.append(ins)
            bb.instructions = newlist


def build(debug=False):
    nc = bass.Bass()
    dt = nc.dram_tensor
    xT = dt("xT", [HIDDEN, S], BF16, kind="ExternalInput")
    Wd = dt("Wd", [HIDDEN, KV_C + Q_C], BF16, kind="ExternalInput")
    bd = dt("bd", [P, NMD], F32, kind="ExternalInput")
    Wku = dt("Wku", [KV_C, NH * HEAD_DIM], BF16, kind="ExternalInput")
    bku = dt("bku", [P, 4], F32, kind="ExternalInput")
    Wvu = dt("Wvu", [KV_C, NH * HEAD_DIM], BF16, kind="ExternalInput")
    Wkr = dt("Wkr", [KV_C, NH * ROPE_DIM], BF16, kind="ExternalInput")
    Wkrs = dt("Wkrs", [KV_C, NH * ROPE_DIM], BF16, kind="ExternalInput")
    bkr = dt("bkr", [P, 2], F32, kind="ExternalInput")
    bkrs = dt("bkrs", [P, 2], F32, kind="ExternalInput")
    Wqu = dt("Wqu", [Q_C, NH * HEAD_DIM], BF16, kind="ExternalInput")
    bqu = dt("bqu", [P, 4], F32, kind="ExternalInput")
    Wqr = dt("Wqr", [Q_C, NH * ROPE_DIM], BF16, kind="ExternalInput")
    Wqrs = dt("Wqrs", [Q_C, NH * ROPE_DIM], BF16, kind="ExternalInput")
    bqr = dt("bqr", [P, 2], F32, kind="ExternalInput")
    bqrs = dt("bqrs", [P, 2], F32, kind="ExternalInput")
    Wo = dt("Wo", [NH * HEAD_DIM, HIDDEN], BF16, kind="ExternalInput")
    cos2 = dt("cos2", [P, S], BF16, kind="ExternalInput")
    sin2 = dt("sin2", [P, S], BF16, kind="ExternalInput")
    tri = dt("tri", [P, P], F32, kind="ExternalInput")
    outT = dt("outT", [HIDDEN, S], F32, kind="ExternalOutput")
    if debug:
        dbg_kvq = dt("dbg_kvq", [HIDDEN, S], BF16, kind="ExternalOutput")
        dbg_kc = dt("dbg_kc", [NH * HEAD_DIM, S], BF16, kind="ExternalOutput")
        dbg_kr = dt("dbg_kr", [2 * P, S], BF16, kind="ExternalOutput")
        dbg_qc = dt("dbg_qc", [NH * HEAD_DIM, S], BF16, kind="ExternalOutput")
        dbg_qr = dt("dbg_qr", [2 * P, S], BF16, kind="ExternalOutput")
        dbg_v = dt("dbg_v", [S, NH * HEAD_DIM], BF16, kind="ExternalOutput")

    NSC = S // SC  # 4 free-dim chunks

    with TileContext(nc) as tc:
        with (
            tc.tile_pool(name="const", bufs=1) as pc,
            tc.tile_pool(name="kvq", bufs=1) as pkvq,
        ):
            # --- constants (gpsimd DMA queue; keep sync queue free for xT) ---
            cos_sb = pc.tile([P, S], BF16)
            sin_sb = pc.tile([P, S], BF16)
            tri_sb = pc.tile([P, P], F32)
            nc.gpsimd.dma_start(tri_sb[:], tri[:])
            bd_sb = pc.tile([P, NMD], F32)
            nc.gpsimd.dma_start(bd_sb[:], bd[:])
            bku_sb = pc.tile([P, 4], F32)
            nc.gpsimd.dma_start(bku_sb[:], bku[:])
            bkr_sb = pc.tile([P, 2], F32)
            nc.gpsimd.dma_start(bkr_sb[:], bkr[:])
            bkrs_sb = pc.tile([P, 2], F32)
            nc.gpsimd.dma_start(bkrs_sb[:], bkrs[:])
            bqu_sb = pc.tile([P, 4], F32)
            nc.gpsimd.dma_start(bqu_sb[:], bqu[:])
            bqr_sb = pc.tile([P, 2], F32)
            nc.gpsimd.dma_start(bqr_sb[:], bqr[:])
            bqrs_sb = pc.tile([P, 2], F32)
            nc.gpsimd.dma_start(bqrs_sb[:], bqrs[:])
            ones_mat = pc.tile([P, P], BF16)
            nc.vector.memset(ones_mat[:], 1.0)
            ones_row = pc.tile([1, P], BF16)
            nc.vector.memset(ones_row[:], 1.0)

            kvq_sb = pkvq.tile([P, NKT, S], BF16)

            # ---------------- phase 1: down projection ----------------
            with (
                tc.tile_pool(name="p1", bufs=1) as p1,
                tc.tile_pool(name="p1w", bufs=3) as p1w,
                tc.tile_pool(name="ps1", bufs=4, space="PSUM") as ps1,
            ):
                xTr = xT.rearrange("(t p) s -> p t s", p=P)
                xt_tiles = []
                for k in range(NKT):
                    t = p1.tile([P, S], BF16, tag=f"xt{k}")
                    nc.sync.dma_start(t[:], xTr[:, k, :])
                    xt_tiles.append(t)
                for m in range(NMD):
                    wd_t = p1w.tile([P, NKT, P], BF16, tag="wd")
                    nc.gpsimd.dma_start(
                        wd_t[:],
                        Wd[:, m * P : (m + 1) * P].rearrange(
                            "(t p) m -> p t m", p=P
                        ),
                    )
                    for s in range(NSC):
                        ps = ps1.tile([P, SC], F32, tag="mm")
                        for k in range(NKT):
                            nc.tensor.matmul(
                                ps[:],
                                wd_t[:, k, :],
                                xt_tiles[k][:, s * SC : (s + 1) * SC],
                                start=(k == 0),
                                stop=(k == NKT - 1),
                            )
                        nc.vector.tensor_scalar_add(
                            kvq_sb[:, m, s * SC : (s + 1) * SC],
                            ps[:],
                            bd_sb[:, m : m + 1],
                        )
                nc.gpsimd.dma_start(cos_sb[:], cos2[:])
                nc.gpsimd.dma_start(sin_sb[:], sin2[:])

            if debug:
                nc.sync.dma_start(
                    dbg_kvq.rearrange("(t p) s -> p t s", p=P), kvq_sb[:]
                )

            # ------------- phase 2: up projections + rope -------------
            with tc.tile_pool(name="qkv", bufs=1) as pq:
                kc_sb = pq.tile([P, NH, S], BF16)
                kr_sb = pq.tile([P, 2, S], BF16)
                qc_sb = pq.tile([P, NH, S], BF16)
                qr_sb = pq.tile([P, 2, S], BF16)
                v_sb = pq.tile([P, S // P, NH * HEAD_DIM], BF16)

                with (
                    tc.tile_pool(name="p2w", bufs=2) as p2w,
                    tc.tile_pool(name="p2t", bufs=3) as p2t,
                    tc.tile_pool(name="ps2", bufs=4, space="PSUM") as ps2,
                ):
                    # K_c^T: 4 chunks of 128 head-features
                    for m in range(NH):
                        wt = p2w.tile([P, 4, P], BF16, tag="wku")
                        nc.sync.dma_start(
                            wt[:],
                            Wku[:, m * P : (m + 1) * P].rearrange(
                                "(t p) m -> p t m", p=P
                            ),
                        )
                        for s in range(NSC):
                            ps = ps2.tile([P, SC], F32, tag="mm")
                            for k in range(4):
                                nc.tensor.matmul(
                                    ps[:],
                                    wt[:, k, :],
                                    kvq_sb[:, k, s * SC : (s + 1) * SC],
                                    start=(k == 0),
                                    stop=(k == 3),
                                )
                            nc.vector.tensor_scalar_add(
                                kc_sb[:, m, s * SC : (s + 1) * SC],
                                ps[:],
                                bku_sb[:, m : m + 1],
                            )
                    # V token-major
                    wv_t = p2w.tile([P, 4, NH * HEAD_DIM], BF16, tag="wvu")
                    nc.sync.dma_start(
                        wv_t[:], Wvu.rearrange("(t p) m -> p t m", p=P)
                    )
                    for t in range(S // P):
                        ps = ps2.tile([P, NH * HEAD_DIM], F32, tag="mm")
                        for k in range(4):
                            nc.tensor.matmul(
                                ps[:],
                                kvq_sb[:, k, t * P : (t + 1) * P],
                                wv_t[:, k, :],
                                start=(k == 0),
                                stop=(k == 3),
                            )
                        nc.vector.tensor_copy(v_sb[:, t, :], ps[:])

                    # K rope (swapped-weight trick), chunks of 2 heads
                    for m in range(2):
                        wa = p2w.tile([P, 4, P], BF16, tag="wkr")
                        nc.sync.dma_start(
                            wa[:],
                            Wkr[:, m * P : (m + 1) * P].rearrange(
                                "(t p) m -> p t m", p=P
                            ),
                        )
                        wb = p2w.tile([P, 4, P], BF16, tag="wkrs")
                        nc.sync.dma_start(
                            wb[:],
                            Wkrs[:, m * P : (m + 1) * P].rearrange(
                                "(t p) m -> p t m", p=P
                            ),
                        )
                        for s in range(NSC):
                            sl = slice(s * SC, (s + 1) * SC)
                            psA = ps2.tile([P, SC], F32, tag="mm")
                            for k in range(4):
                                nc.tensor.matmul(
                                    psA[:], wa[:, k, :], kvq_sb[:, k, sl],
                                    start=(k == 0), stop=(k == 3),
                                )
                            psB = ps2.tile([P, SC], F32, tag="mm")
                            for k in range(4):
                                nc.tensor.matmul(
                                    psB[:], wb[:, k, :], kvq_sb[:, k, sl],
                                    start=(k == 0), stop=(k == 3),
                                )
                            tA = p2t.tile([P, SC], F32, tag="ropeA")
                            nc.vector.tensor_scalar_add(
                                tA[:], psA[:], bkr_sb[:, m : m + 1]
                            )
                            tB = p2t.tile([P, SC], F32, tag="ropeB")
                            nc.vector.tensor_scalar_add(
                                tB[:], psB[:], bkrs_sb[:, m : m + 1]
                            )
                            nc.vector.tensor_tensor(
                                tA[:], tA[:], cos_sb[:, sl],
                                mybir.AluOpType.mult,
                            )
                            nc.vector.tensor_tensor(
                                tB[:], tB[:], sin_sb[:, sl],
                                mybir.AluOpType.mult,
                            )
                            nc.vector.tensor_tensor(
                                kr_sb[:, m, sl], tA[:], tB[:],
                                mybir.AluOpType.add,
                            )

                    # Q_c^T
                    for m in range(NH):
                        wt = p2w.tile([P, 12, P], BF16, tag="wqu")
                        nc.sync.dma_start(
                            wt[:],
                            Wqu[:, m * P : (m + 1) * P].rearrange(
                                "(t p) m -> p t m", p=P
                            ),
                        )
                        for s in range(NSC):
                            ps = ps2.tile([P, SC], F32, tag="mm")
                            for k in range(12):
                                nc.tensor.matmul(
                                    ps[:],
                                    wt[:, k, :],
                                    kvq_sb[:, 4 + k, s * SC : (s + 1) * SC],
                                    start=(k == 0),
                                    stop=(k == 11),
                                )
                            nc.vector.tensor_scalar_add(
                                qc_sb[:, m, s * SC : (s + 1) * SC],
                                ps[:],
                                bqu_sb[:, m : m + 1],
                            )
                    # Q rope
                    for m in range(2):
                        wa = p2w.tile([P, 12, P], BF16, tag="wqr")
                        nc.sync.dma_start(
                            wa[:],
                            Wqr[:, m * P : (m + 1) * P].rearrange(
                                "(t p) m -> p t m", p=P
                            ),
                        )
                        wb = p2w.tile([P, 12, P], BF16, tag="wqrs")
                        nc.sync.dma_start(
                            wb[:],
                            Wqrs[:, m * P : (m + 1) * P].rearrange(
                                "(t p) m -> p t m", p=P
                            ),
                        )
                        for s in range(NSC):
                            sl = slice(s * SC, (s + 1) * SC)
                            psA = ps2.tile([P, SC], F32, tag="mm")
                            for k in range(12):
                                nc.tensor.matmul(
                                    psA[:], wa[:, k, :], kvq_sb[:, 4 + k, sl],
                                    start=(k == 0), stop=(k == 11),
                                )
                            psB = ps2.tile([P, SC], F32, tag="mm")
                            for k in range(12):
                                nc.tensor.matmul(
                                    psB[:], wb[:, k, :], kvq_sb[:, 4 + k, sl],
                                    start=(k == 0), stop=(k == 11),
                                )
                            tA = p2t.tile([P, SC], F32, tag="ropeA")
                            nc.vector.tensor_scalar_add(
                                tA[:], psA[:], bqr_sb[:, m : m + 1]
                            )
                            tB = p2t.tile([P, SC], F32, tag="ropeB")
                            nc.vector.tensor_scalar_add(
                                tB[:], psB[:], bqrs_sb[:, m : m + 1]
                            )
                            nc.vector.tensor_tensor(
                                tA[:], tA[:], cos_sb[:, sl],
                                mybir.AluOpType.mult,
                            )
                            nc.vector.tensor_tensor(
                                tB[:], tB[:], sin_sb[:, sl],
                                mybir.AluOpType.mult,
                            )
                            nc.vector.tensor_tensor(
                                qr_sb[:, m, sl], tA[:], tB[:],
                                mybir.AluOpType.add,
                            )

                if debug:
                    nc.sync.dma_start(
                        dbg_kc.rearrange("(t p) s -> p t s", p=P), kc_sb[:]
                    )
                    nc.sync.dma_start(
                        dbg_kr.rearrange("(t p) s -> p t s", p=P), kr_sb[:]
                    )
                    nc.sync.dma_start(
                        dbg_qc.rearrange("(t p) s -> p t s", p=P), qc_sb[:]
                    )
                    nc.sync.dma_start(
                        dbg_qr.rearrange("(t p) s -> p t s", p=P), qr_sb[:]
                    )
                    nc.sync.dma_start(
                        dbg_v.rearrange("(t p) d -> p t d", p=P), v_sb[:]
                    )

                # ---------- phase 3: attention + inline out-proj ----------
                with (
                    tc.tile_pool(name="at", bufs=8) as pat,
                    tc.tile_pool(name="atx", bufs=2) as patx,
                    tc.tile_pool(name="att", bufs=2) as patt,
                    tc.tile_pool(name="out", bufs=3) as pout,
                    tc.tile_pool(name="ow", bufs=3) as pow_,
                    tc.tile_pool(name="ps_sc", bufs=2, space="PSUM") as ps_sc,
                    tc.tile_pool(name="ps_acc", bufs=2, space="PSUM") as ps_acc,
                    tc.tile_pool(name="ps_m", bufs=2, space="PSUM") as ps_m,
                ):
                    for qc in range(NSC):
                        qsl = slice(qc * SC, (qc + 1) * SC)
                        nkb = 4 * qc + 4
                        ctx_q = patx.tile([P, NH, SC], BF16, tag="ctx")
                        for h in range(NH):
                            hc = h // 2
                            hp = (h % 2) * ROPE_DIM
                            psum_ctx = ps_acc.tile([P, SC], F32, tag="ctx")
                            psum_sum = ps_acc.tile([P, SC], F32, tag="sum")
                            for kb in range(nkb):
                                ksl = slice(kb * P, (kb + 1) * P)
                                diag = kb >= 4 * qc
                                c = (kb - 4 * qc) * P if diag else 0
                                qs0 = qc * SC + c
                                ps = ps_sc.tile([P, SC], F32, tag="sc")
                                nc.tensor.matmul(
                                    ps[:, c:],
                                    kc_sb[:, h, ksl],
                                    qc_sb[:, h, qs0 : (qc + 1) * SC],
                                    start=True, stop=False,
                                )
                                nc.tensor.matmul(
                                    ps[:, c:],
                                    kr_sb[hp : hp + ROPE_DIM, hc, ksl],
                                    qr_sb[hp : hp + ROPE_DIM, hc,
                                          qs0 : (qc + 1) * SC],
                                    start=False, stop=True,
                                )
                                probs = pat.tile([P, SC], BF16, tag="probs")
                                if diag:
                                    nc.vector.tensor_tensor(
                                        ps[:, c : c + P],
                                        ps[:, c : c + P],
                                        tri_sb[:],
                                        mybir.AluOpType.add,
                                    )
                                nc.scalar.activation(
                                    probs[:, c:], ps[:, c:], AF.Exp,
                                    scale=SCALE,
                                )
                                nc.tensor.matmul(
                                    psum_sum[:, c:], ones_mat[:],
                                    probs[:, c:],
                                    start=(kb == 0), stop=(kb == nkb - 1),
                                )
                                nc.tensor.matmul(
                                    psum_ctx[:, c:],
                                    v_sb[:, kb, h * P : (h + 1) * P],
                                    probs[:, c:],
                                    start=(kb == 0), stop=(kb == nkb - 1),
                                )
                            sums_f = patt.tile([1, SC], F32, tag="sums")
                            nc.scalar.copy(sums_f[:], psum_sum[0:1, :])
                            r = patt.tile([1, SC], F32, tag="recip")
                            nc.vector.reciprocal(r[:], sums_f[:])
                            r16 = patt.tile([1, SC], BF16, tag="r16")
                            nc.vector.tensor_copy(r16[:], r[:])
                            psb = ps_m.tile([P, SC], F32, tag="m")
                            nc.tensor.matmul(
                                psb[:], ones_row[:], r16[:],
                                start=True, stop=True,
                            )
                            rbc = patt.tile([P, SC], BF16, tag="rbc")
                            nc.scalar.copy(rbc[:], psb[:])
                            nc.vector.tensor_tensor(
                                ctx_q[:, h, :], psum_ctx[:], rbc[:],
                                mybir.AluOpType.mult,
                            )

                        # out-projection for this q-chunk
                        for m in range(NMD):
                            wo_t = pow_.tile([P, NH, P], BF16, tag="wo")
                            nc.sync.dma_start(
                                wo_t[:],
                                Wo[:, m * P : (m + 1) * P].rearrange(
                                    "(t p) m -> p t m", p=P
                                ),
                            )
                            ps = ps_m.tile([P, SC], F32, tag="m")
                            for k in range(NH):
                                nc.tensor.matmul(
                                    ps[:],
                                    wo_t[:, k, :],
                                    ctx_q[:, k, :],
                                    start=(k == 0),
                                    stop=(k == NH - 1),
                                )
                            og = pout.tile([P, SC], F32, tag="og")
                            nc.scalar.copy(og[:], ps[:])
                            nc.sync.dma_start(
                                outT[m * P : (m + 1) * P,
                                     qc * SC : (qc + 1) * SC],
                                og[:],
                            )
    _split_waits(nc)
    return nc


def _swap_pairs(w):
    """(..., 2i) <- -(..., 2i+1); (..., 2i+1) <- (..., 2i) along last axis."""
    out = np.empty_like(w)
    out[..., 0::2] = -w[..., 1::2]
    out[..., 1::2] = w[..., 0::2]
    return out


def _col_bias(b, nm):
    """[nm*128] -> [128, nm] (column m = bias for feature chunk m)."""
    return np.ascontiguousarray(b.reshape(nm, P).T).astype(np.float32)


_NC = None


def kernel(**inputs):
    global _NC
    inp = {k: np.asarray(v) for k, v in inputs.items()}
    x = inp["x"].astype(np.float32)

    Wd_full = np.concatenate(
        [inp["kv_down_w"], inp["query_down_w"]], axis=1
    ).astype(BF)
    bd_full = np.concatenate([inp["kv_down_b"], inp["query_down_b"]])

    pos = np.arange(S, dtype=np.float64)
    inv = 1.0 / (10000.0 ** (np.arange(0, ROPE_DIM, 2, np.float64) / ROPE_DIM))
    ang = pos[None, :] * inv[:, None]          # [32, S]
    idx = (np.arange(P) % ROPE_DIM) // 2       # row -> freq index
    cos2 = np.cos(ang)[idx].astype(BF)
    sin2 = np.sin(ang)[idx].astype(BF)
    tri = np.where(
        np.arange(P)[None, :] >= np.arange(P)[:, None], 0.0, NEG
    ).astype(np.float32)

    in_maps = []
    for c in range(8):
        b, g = c // 4, c % 4
        h0 = g * NH
        csl = slice(h0 * HEAD_DIM, (h0 + NH) * HEAD_DIM)
        rsl = slice(h0 * ROPE_DIM, (h0 + NH) * ROPE_DIM)
        wkr = inp["key_rope_w"][:, rsl].astype(np.float32)
        wqr = inp["query_rope_w"][:, rsl].astype(np.float32)
        bkr = inp["key_rope_b"][rsl].astype(np.float32)
        bqr = inp["query_rope_b"][rsl].astype(np.float32)
        in_maps.append(
            {
                "xT": np.ascontiguousarray(x[b].T).astype(BF),
                "Wd": Wd_full,
                "bd": _col_bias(bd_full, NMD),
                "Wku": inp["key_up_w"][:, csl].astype(BF),
                "bku": _col_bias(inp["key_up_b"][csl], 4),
                "Wvu": inp["value_up_w"][:, csl].astype(BF),
                "Wkr": wkr.astype(BF),
                "Wkrs": _swap_pairs(wkr).astype(BF),
                "bkr": _col_bias(bkr, 2),
                "bkrs": _col_bias(_swap_pairs(bkr), 2),
                "Wqu": inp["query_up_w"][:, csl].astype(BF),
                "bqu": _col_bias(inp["query_up_b"][csl], 4),
                "Wqr": wqr.astype(BF),
                "Wqrs": _swap_pairs(wqr).astype(BF),
                "bqr": _col_bias(bqr, 2),
                "bqrs": _col_bias(_swap_pairs(bqr), 2),
                "Wo": inp["out_w"][csl, :].astype(BF),
                "cos2": cos2,
                "sin2": sin2,
                "tri": tri,
            }
        )

    if _NC is None:
        _NC = build()
    res = run_bass_kernel_spmd(_NC, in_maps, core_ids=list(range(8)))

    corr = (
        inp["value_up_b"].astype(np.float32) @ inp["out_w"].astype(np.float32)
        + inp["out_b"].astype(np.float32)
    )
    out = np.empty((B, S, HIDDEN), np.float32)
    for b in range(B):
        acc = res.results[b * 4]["outT"].copy()
        for g in range(1, 4):
            acc += res.results[b * 4 + g]["outT"]
        out[b] = acc.T + corr[None, :]
    return out

